# revision 10
# baseline (speedup 1.0000x reference)
"""BiLinearInteractionLayer (bilinear_type='all') Trainium2 Bass kernel.

Contract: kernel(inputs=[2048,40,64] f32, w=[64,64] f32) -> [2048, 49920] f32,
matching

    xw  = einsum('bfd,de->bfe', inputs, w)
    p   = xw[:, I, :] * inputs[:, J, :]   # (I, J) = triu_indices(40, k=1)
    out = p.reshape(B, -1)

Data-parallel over 8 NeuronCores: batch 2048 -> 8 x 256, W replicated.
Per core, each 128-row batch tile:
  - x tile [128, 2560] DMAs to SBUF (tail fields first so the small tail
    pair-blocks can start immediately)
  - PE transposes field pairs ([128,128] -> PSUM), ACT copies to SBUF,
    PE matmuls against replicated W (one PSUM tile per matmul), ACT
    copies xw to SBUF
  - per leading field i: one DVE broadcast-multiply of xw[:, i-block]
    against x[:, j>i], then one DMA of the [128, (39-i)*64] block
    straight to its contiguous slice of the output row
The kernel runs at the HBM-per-core write wall (~360 GB/s aggregate,
51 MB of output per core). SDMA engine 15 is ~15% slower than engines
0-14, so under the even 16-way round-robin it straggles ~7 us after the
other engines drain. To let it finish with the pack, the last pair-blocks
(i=34..38, 960 cols) are staged and written at the very end of the queue
via partial-partition DMAs (92/28/4/4 rows), which the DGE routes to
engines 0-13 only.
"""

import numpy as np
from contextlib import ExitStack

import concourse.bass as bass  # noqa: F401  (registers engines)
import concourse.bacc as bacc
import concourse.tile as tile
import concourse.mybir as mybir
from concourse.bass_utils import run_bass_kernel_spmd

B = 2048
F = 40
D = 64
NCORES = 8
BS = B // NCORES                   # 256 rows per core
PAIRS = F * (F - 1) // 2           # 780
OUT_W = PAIRS * D                  # 49920
FD = F * D                         # 2560
DT = mybir.dt.float32

BLOCK_LEN = [F - 1 - i for i in range(F - 1)]
BLOCK_OFF = np.concatenate([[0], np.cumsum(BLOCK_LEN)[:-1]]).tolist()

# tail field-pairs first: their pair-blocks are small and depend only on
# the tail x chunk, so the output DMA stream starts earliest. Splitting at
# field 24 gives ~3.4 MB of early small-block writes (i=24..33) to keep the
# DMA engines fed until the big blocks start flowing.
SPLIT_F = 24
FP_ORDER = list(range(SPLIT_F // 2, F // 2)) + list(range(SPLIT_F // 2))

TAIL_I0 = 34                       # blocks 34..38 staged, written at the end
TAIL_C0 = BLOCK_OFF[TAIL_I0] * D   # output col of the staged tail
TAIL_COLS = OUT_W - TAIL_C0        # 960

_CACHE = {}


def _build(bs: int):
    assert bs % 128 == 0
    ntiles = bs // 128
    nc = bacc.Bacc("TRN2", target_bir_lowering=False, debug=False)

    x_dram = nc.dram_tensor("x", [bs, F, D], DT, kind="ExternalInput").ap()
    w_dram = nc.dram_tensor("w", [D, D], DT, kind="ExternalInput").ap()
    id_dram = nc.dram_tensor("ident", [128, 128], DT, kind="ExternalInput").ap()
    out_dram = nc.dram_tensor("out", [bs, OUT_W], DT, kind="ExternalOutput").ap()

    x_flat = x_dram.rearrange("b f d -> b (f d)")
    c0 = SPLIT_F * D

    with tile.TileContext(nc) as tc, ExitStack() as ctx:
        const_pool = ctx.enter_context(tc.tile_pool(name="const", bufs=1))
        x_pool = ctx.enter_context(tc.tile_pool(name="x", bufs=2))
        xw_pool = ctx.enter_context(tc.tile_pool(name="xw", bufs=2))
        tr_pool = ctx.enter_context(tc.tile_pool(name="tr", bufs=3))
        stage_pool = ctx.enter_context(tc.tile_pool(name="stage", bufs=10))
        tail_pool = ctx.enter_context(tc.tile_pool(name="tailst", bufs=ntiles))
        psum_tr = ctx.enter_context(tc.tile_pool(name="psum_tr", bufs=2, space="PSUM"))
        psum_mm = ctx.enter_context(tc.tile_pool(name="psum_mm", bufs=4, space="PSUM"))

        ident = const_pool.tile([128, 128], DT)
        nc.scalar.dma_start(ident[:], id_dram)
        # W on both partition halves so the two per-pair matmuls read lhsT
        # and rhs from the same base partition
        w_sb = const_pool.tile([128, D], DT)
        nc.scalar.dma_start(w_sb[0:D, :], w_dram)
        nc.scalar.dma_start(w_sb[D:128, :], w_dram)

        x_tiles = []
        for t in range(ntiles):
            b0 = t * 128
            x_t = x_pool.tile([128, FD], DT)
            x_tiles.append(x_t)
            # tail fields first (sync ring), rest on the scalar ring
            nc.sync.dma_start(x_t[:, c0:FD], x_flat[b0 : b0 + 128, c0:FD])
            nc.scalar.dma_start(x_t[:, 0:c0], x_flat[b0 : b0 + 128, 0:c0])

        tails = []
        for t in range(ntiles):
            b0 = t * 128
            x_t = x_tiles[t]
            xw_t = xw_pool.tile([128, FD], DT)
            st_tail = tail_pool.tile([128, TAIL_COLS], DT)
            tails.append((b0, st_tail))
            for fp in FP_ORDER:
                tr_ps = psum_tr.tile([128, 128], DT)
                nc.tensor.transpose(
                    tr_ps[:], x_t[:, fp * 128 : (fp + 1) * 128], ident[:]
                )
                tr_sb = tr_pool.tile([128, 128], DT)
                nc.scalar.copy(tr_sb[:], tr_ps[:])
                for h in range(2):
                    i = 2 * fp + h
                    mm = psum_mm.tile([128, D], DT, tag="mm")
                    nc.tensor.matmul(
                        mm[:],
                        tr_sb[h * D : (h + 1) * D, :],
                        w_sb[h * D : (h + 1) * D, :],
                        start=True,
                        stop=True,
                    )
                    nc.scalar.copy(xw_t[:, i * D : (i + 1) * D], mm[:])
                for h in range(2):
                    i = 2 * fp + h
                    if i > F - 2 or i >= TAIL_I0:
                        continue  # field 39 never leads; tail blocks deferred
                    jn = F - 1 - i
                    in0 = (
                        xw_t[:, i * D : (i + 1) * D]
                        .unsqueeze(1)
                        .broadcast_to([128, jn, D])
                    )
                    in1 = x_t[:, (i + 1) * D : FD].rearrange(
                        "p (j d) -> p j d", d=D
                    )
                    st = stage_pool.tile([128, jn * D], DT)
                    nc.vector.tensor_mul(
                        st[:].rearrange("p (j d) -> p j d", d=D), in0, in1
                    )
                    nc.sync.dma_start(
                        out_dram[
                            b0 : b0 + 128,
                            BLOCK_OFF[i] * D : (BLOCK_OFF[i] + jn) * D,
                        ],
                        st[:],
                    )
            # staged tail multiplies last so they don't interrupt the early
            # DVE stream; written at the very end via partials so slow
            # engine 15 can finish with the pack
            for i in range(TAIL_I0, F - 1):
                jn = F - 1 - i
                lo = BLOCK_OFF[i] * D - TAIL_C0
                nc.vector.tensor_mul(
                    st_tail[:, lo : lo + jn * D].rearrange("p (j d) -> p j d", d=D),
                    xw_t[:, i * D : (i + 1) * D]
                    .unsqueeze(1)
                    .broadcast_to([128, jn, D]),
                    x_t[:, (i + 1) * D : FD].rearrange("p (j d) -> p j d", d=D),
                )

        # endgame: staged tails via partial-partition DMAs (engines 0-13)
        for b0, st_tail in tails:
            nc.sync.dma_start(
                out_dram[b0 + 0 : b0 + 92, TAIL_C0:OUT_W], st_tail[0:92, :]
            )
            nc.sync.dma_start(
                out_dram[b0 + 96 : b0 + 124, TAIL_C0:OUT_W], st_tail[96:124, :]
            )
            nc.sync.dma_start(
                out_dram[b0 + 92 : b0 + 96, TAIL_C0:OUT_W], st_tail[92:96, :]
            )
            nc.sync.dma_start(
                out_dram[b0 + 124 : b0 + 128, TAIL_C0:OUT_W], st_tail[124:128, :]
            )

    nc.compile()
    return nc


def _get_nc(bs: int):
    if bs not in _CACHE:
        _CACHE[bs] = _build(bs)
    return _CACHE[bs]


def _run(inputs: np.ndarray, w: np.ndarray, trace: bool = False):
    inputs = np.ascontiguousarray(inputs, dtype=np.float32)
    w = np.ascontiguousarray(w, dtype=np.float32)
    assert inputs.shape == (B, F, D) and w.shape == (D, D)
    nc = _get_nc(BS)
    ident = np.eye(128, dtype=np.float32)
    in_maps = [
        {"x": inputs[c * BS : (c + 1) * BS], "w": w, "ident": ident}
        for c in range(NCORES)
    ]
    res = run_bass_kernel_spmd(nc, in_maps, list(range(NCORES)), trace=trace)
    out = np.concatenate([res.results[c]["out"] for c in range(NCORES)], axis=0)
    return out, res


def kernel(inputs: np.ndarray, w: np.ndarray) -> np.ndarray:
    out, _ = _run(inputs, w)
    return out
